# revision 1
# baseline (speedup 1.0000x reference)
"""BEiT-style attention (B=64, N=197, C=768, H=12, rel-pos bias) on 8 TRN2 cores.

Data-parallel over batch: 8 batch items per core, no collectives. Per-core
Bass/Tile kernel computes qkv projection, attention with relative position
bias, softmax, and output projection, all matmuls in bf16 with fp32 PSUM
accumulation.

Layout choices (all chosen to avoid on-chip transposes of activations):
  - x is host-cast to bf16 and DMA-xbar-transposed into xT [C, M] on load.
  - qkv computed transposed: qkT [j, m] (j = qkv row), so per-head qT/kT
    [hd, n] slices are direct row-slices. v computed un-transposed [m, d]
    with per-batch 128-aligned token chunks.
  - Attention runs in the S^T layout: S^T[m, n] = k @ q^T (keys on
    partitions). Rel-pos bias is pre-filled into PSUM with an
    identity-matmul, S matmuls accumulate on top. One ACT pass does
    exp -> E^T bf16.
  - Softmax denominators come from a ones-matmul over E^T broadcast to 64
    rows; O^T = v^T E^T lands in the transposed layout the proj matmul
    needs as lhsT. A single DVE reciprocal + multiply normalizes while
    evacuating PSUM.
  - v_bias is algebraically folded into the proj bias on the host
    (softmax rows sum to 1), and the proj bias enters via a K=1 ones-row
    matmul. q_bias/scale are folded into the qk weight / ACT evacuation.
"""

import numpy as np
import ml_dtypes

import concourse.bass as bass
import concourse.mybir as mybir
import concourse.tile as tile
from concourse import bacc
from concourse.bass_utils import run_bass_kernel_spmd
from concourse.masks import make_identity

BF16 = ml_dtypes.bfloat16
F32 = mybir.dt.float32
BF = mybir.dt.bfloat16

B, N, C = 64, 197, 768
H, HD = 12, 64
NCORES = 8
BL = B // NCORES            # 8 batches per core
M = BL * N                  # 1576 tokens per core
MPAD = 1584                 # 16-aligned for DMA transpose
SCALE = HD ** -0.5
# per-batch token chunks (psum partition dim must be 128-aligned per batch)
BCH = [(0, 128), (128, 69)]
# m free-dim chunks for the qkT matmul (psum bank = 512 f32)
MCH = [(0, 512), (512, 512), (1024, 512), (1536, 40)]

_NC = None


def _build():
    nc = bacc.Bacc("TRN2", target_bir_lowering=False, debug=False)

    x = nc.dram_tensor("x", [C, MPAD], BF, kind="ExternalInput")
    wqk = nc.dram_tensor("wqk", [C, 2 * C], BF, kind="ExternalInput")
    wv = nc.dram_tensor("wv", [C, C], BF, kind="ExternalInput")
    wp = nc.dram_tensor("wp", [C, C], BF, kind="ExternalInput")
    rpbt = nc.dram_tensor("rpbt", [2, 6, 128, 512], BF, kind="ExternalInput")
    qb = nc.dram_tensor("qb", [128, 6], F32, kind="ExternalInput")
    pb = nc.dram_tensor("pb", [1, C], BF, kind="ExternalInput")
    out = nc.dram_tensor("out", [M, C], F32, kind="ExternalOutput")

    Ident = mybir.ActivationFunctionType.Identity
    Copy = mybir.ActivationFunctionType.Copy
    Exp = mybir.ActivationFunctionType.Exp

    with tile.TileContext(nc) as tc:
        with (
            tc.tile_pool(name="persist", bufs=1) as P,
            tc.tile_pool(name="et", bufs=3) as et_pool,
            tc.tile_pool(name="rcp", bufs=3) as r_pool,
            tc.tile_pool(name="ob", bufs=3) as ob_pool,
            tc.tile_pool(name="mm", bufs=8, space="PSUM") as mm,
        ):
            # ---- constants / inputs to SBUF ----
            xT = [P.tile([128, MPAD], BF, tag=f"xt{t}", name=f"xt{t}") for t in range(6)]
            for t in range(6):
                nc.sync.dma_start(xT[t][:, :], x[128 * t : 128 * (t + 1), :])
            wqk_sb = [P.tile([128, 2 * C], BF, tag=f"wqk{t}", name=f"wqk{t}") for t in range(6)]
            for t in range(6):
                nc.sync.dma_start(wqk_sb[t][:, :], wqk[128 * t : 128 * (t + 1), :])
            qb_sb = P.tile([128, 6], F32, tag="qb")
            nc.sync.dma_start(qb_sb[:, :], qb[:, :])
            wv_sb = [P.tile([128, C], BF, tag=f"wv{t}", name=f"wv{t}") for t in range(6)]
            for t in range(6):
                nc.sync.dma_start(wv_sb[t][:, :], wv[128 * t : 128 * (t + 1), :])
            wp_sb = [P.tile([128, C], BF, tag=f"wp{t}", name=f"wp{t}") for t in range(6)]
            for t in range(6):
                nc.sync.dma_start(wp_sb[t][:, :], wp[128 * t : 128 * (t + 1), :])
            pb_sb = P.tile([1, C], BF, tag="pb")
            nc.sync.dma_start(pb_sb[:, :], pb[:, :])
            rpbt_sb = [
                [P.tile([128, 512], BF, tag=f"rpb{c}_{p}", name=f"rpb{c}_{p}") for p in range(6)]
                for c in range(2)
            ]
            for c in range(2):
                for p in range(6):
                    nc.sync.dma_start(rpbt_sb[c][p][:, :], rpbt[c, p, :, :])

            ident = P.tile([128, 128], BF, tag="ident")
            make_identity(nc, ident[:, :])
            ones64 = P.tile([128, 64], BF, tag="ones64")
            nc.gpsimd.memset(ones64[:, :], 1.0)
            onesrow = P.tile([1, 128], BF, tag="onesrow")
            nc.gpsimd.memset(onesrow[:, :], 1.0)

            # ---- stage B: qkT[j, m] = (Wqk^T)^T @ x^T  (q rows pre-scaled) ----
            qkT = [P.tile([128, MPAD], BF, tag=f"qkt{j}", name=f"qkt{j}") for j in range(12)]
            for j in range(12):
                for m0, mw in MCH:
                    ps = mm.tile([128, 512], F32, tag="mm")
                    for ct in range(6):
                        nc.tensor.matmul(
                            ps[:, :mw],
                            lhsT=wqk_sb[ct][:, 128 * j : 128 * (j + 1)],
                            rhs=xT[ct][:, m0 : m0 + mw],
                            start=(ct == 0),
                            stop=(ct == 5),
                        )
                    if j < 6:  # q rows: add scaled bias during evacuation
                        nc.scalar.activation(
                            qkT[j][:, m0 : m0 + mw],
                            ps[:, :mw],
                            Ident,
                            bias=qb_sb[:, j : j + 1],
                        )
                    else:  # k rows: plain copy/cast
                        nc.scalar.activation(qkT[j][:, m0 : m0 + mw], ps[:, :mw], Copy)

            # ---- stage C: v[m, d] per-batch aligned chunks ----
            v_sb = [P.tile([128, 2 * C], BF, tag=f"v{b}", name=f"v{b}") for b in range(BL)]
            for b in range(BL):
                for ch, (off, mr) in enumerate(BCH):
                    for f0, fw in [(0, 512), (512, 256)]:
                        ps = mm.tile([128, 512], F32, tag="mm")
                        for ct in range(6):
                            nc.tensor.matmul(
                                ps[:mr, :fw],
                                lhsT=xT[ct][:, b * N + off : b * N + off + mr],
                                rhs=wv_sb[ct][:, f0 : f0 + fw],
                                start=(ct == 0),
                                stop=(ct == 5),
                            )
                        nc.scalar.activation(
                            v_sb[b][:mr, ch * C + f0 : ch * C + f0 + fw],
                            ps[:mr, :fw],
                            Copy,
                        )

            # ---- stage D: attention per (batch, head-pair) ----
            # S^T runs in per-head PSUM banks: the odd head's lhsT sits at
            # partition base 64, which row-tiles the matmul (PE runs both
            # heads' S matmuls concurrently on disjoint row groups).
            # Concurrent matmuls must drain to different PSUM banks --
            # same-bank overlapping-partition drains fault the device.
            AT = [P.tile([128, MPAD], BF, tag=f"at{p}", name=f"at{p}") for p in range(6)]
            for b in range(BL):
                for p in range(6):
                    psH = [
                        mm.tile([128, 512], F32, tag="mm", name=f"psH{hj}")
                        for hj in range(2)
                    ]
                    # pre-fill rel-pos bias via identity matmuls (serial: base-0)
                    for hj in range(2):
                        for ch, (off, mr) in enumerate(BCH):
                            nc.tensor.matmul(
                                psH[hj][:mr, 256 * ch : 256 * ch + N],
                                lhsT=ident[:mr, :mr],
                                rhs=rpbt_sb[hj][p][:mr, 256 * ch : 256 * ch + N],
                                start=(ch == 0),
                                stop=False,
                            )
                    # S matmuls; ch1 first so each bank's stop covers all 128
                    # partitions; adjacent concurrent pairs hit different banks
                    for ch in (1, 0):
                        off, mr = BCH[ch]
                        for hj in range(2):
                            nc.tensor.matmul(
                                psH[hj][:mr, 256 * ch : 256 * ch + N],
                                lhsT=qkT[6 + p][
                                    64 * hj : 64 * (hj + 1), b * N + off : b * N + off + mr
                                ],
                                rhs=qkT[p][64 * hj : 64 * (hj + 1), b * N : b * N + N],
                                start=False,
                                stop=(ch == 0),
                            )
                    # exp(S^T) -> E^T bf16; [128, 4, N]: index = ch*2 + head
                    et = et_pool.tile([128, 4, N], BF, tag="et")
                    for hj in range(2):
                        for ch, (off, mr) in enumerate(BCH):
                            nc.scalar.activation(
                                et[:mr, 2 * hj + ch, :],
                                psH[hj][:mr, 256 * ch : 256 * ch + N],
                                Exp,
                            )
                    # O^T (cols 0:197) and broadcast denominators (cols 256:453)
                    psOD = mm.tile([128, 512], F32, tag="mm")
                    for hj in range(2):
                        tp = None if hj == 0 else (0, 64)
                        for ch, (off, mr) in enumerate(BCH):
                            nc.tensor.matmul(
                                psOD[64 * hj : 64 * (hj + 1), 0:N],
                                lhsT=v_sb[b][:mr, ch * C + (2 * p + hj) * HD : ch * C + (2 * p + hj + 1) * HD],
                                rhs=et[:mr, 2 * hj + ch, :],
                                start=(ch == 0),
                                stop=False,
                                tile_position=tp,
                            )
                        for ch, (off, mr) in enumerate(BCH):
                            nc.tensor.matmul(
                                psOD[64 * hj : 64 * (hj + 1), 256 : 256 + N],
                                lhsT=ones64[:mr, :],
                                rhs=et[:mr, 2 * hj + ch, :],
                                start=False,
                                stop=(ch == 1),
                                tile_position=tp,
                            )
                    rcp = r_pool.tile([128, N], F32, tag="rcp")
                    nc.vector.reciprocal(rcp[:, :], psOD[:, 256 : 256 + N])
                    nc.vector.tensor_mul(
                        AT[p][:, b * N : b * N + N], psOD[:, 0:N], rcp[:, :]
                    )

            # ---- stage E: out[n, c'] = A @ Wp^T + pb (bias via ones-row) ----
            for b in range(BL):
                for ch, (off, nr) in enumerate(BCH):
                    ob = ob_pool.tile([128, C], F32, tag="ob")
                    for f0, fw in [(0, 512), (512, 256)]:
                        ps = mm.tile([128, 512], F32, tag="mm")
                        for ct in range(6):
                            nc.tensor.matmul(
                                ps[:nr, :fw],
                                lhsT=AT[ct][:, b * N + off : b * N + off + nr],
                                rhs=wp_sb[ct][:, f0 : f0 + fw],
                                start=(ct == 0),
                                stop=False,
                            )
                        nc.tensor.matmul(
                            ps[:nr, :fw],
                            lhsT=onesrow[0:1, :nr],
                            rhs=pb_sb[0:1, f0 : f0 + fw],
                            start=False,
                            stop=True,
                        )
                        nc.scalar.activation(ob[:nr, f0 : f0 + fw], ps[:nr, :fw], Copy)
                    nc.sync.dma_start(
                        out[b * N + off : b * N + off + nr, :], ob[:nr, :]
                    )

    nc.compile()
    return nc


def _host_prep(inputs):
    x = np.asarray(inputs["x"], np.float32)
    qkv_w = np.asarray(inputs["qkv_w"], np.float32)
    q_bias = np.asarray(inputs["q_bias"], np.float32)
    v_bias = np.asarray(inputs["v_bias"], np.float32)
    rel_table = np.asarray(inputs["rel_table"], np.float32)
    proj_w = np.asarray(inputs["proj_w"], np.float32)
    proj_b = np.asarray(inputs["proj_b"], np.float32)
    rel_index = np.asarray(inputs["rel_index"], np.int64)

    wqk_t = qkv_w[: 2 * C].T.copy()
    wqk_t[:, :C] *= SCALE  # fold q scale into weights (exact: power of 2)
    wqk_np = wqk_t.astype(BF16)
    wv_np = np.ascontiguousarray(qkv_w[2 * C :].T).astype(BF16)
    wp_np = np.ascontiguousarray(proj_w.T).astype(BF16)
    qb_np = np.ascontiguousarray((q_bias * SCALE).reshape(6, 128).T).astype(np.float32)
    pb_np = (proj_b + v_bias @ proj_w.T).astype(BF16).reshape(1, C)

    rpb = rel_table[rel_index]              # [N, N, H]
    rpbT = np.transpose(rpb, (2, 1, 0))     # [H, m, n]
    rpbt_np = np.zeros((2, 6, 128, 512), np.float32)
    for p in range(6):
        for hj in range(2):
            h = 2 * p + hj
            rpbt_np[hj, p, 0:128, 0:N] = rpbT[h, 0:128, :]
            rpbt_np[hj, p, 0:69, 256 : 256 + N] = rpbT[h, 128:N, :]
    rpbt_np = rpbt_np.astype(BF16)

    consts = {
        "wqk": wqk_np,
        "wv": wv_np,
        "wp": wp_np,
        "rpbt": rpbt_np,
        "qb": qb_np,
        "pb": pb_np,
    }
    in_maps = []
    for i in range(NCORES):
        xi = x[BL * i : BL * (i + 1)].reshape(M, C)
        xpad = np.zeros((MPAD, C), np.float32)
        xpad[:M] = xi
        in_maps.append({"x": np.ascontiguousarray(xpad.T).astype(BF16), **consts})
    return in_maps


def _run(inputs, trace=False):
    global _NC
    if _NC is None:
        _NC = _build()
    in_maps = _host_prep(inputs)
    res = run_bass_kernel_spmd(_NC, in_maps, core_ids=list(range(NCORES)), trace=trace)
    outs = [
        np.asarray(res.results[i]["out"], np.float32).reshape(BL, N, C)
        for i in range(NCORES)
    ]
    full = np.concatenate(outs, axis=0)
    return full, res


def kernel(**inputs) -> np.ndarray:
    full, _ = _run(inputs, trace=False)
    return full

